# revision 70
# baseline (speedup 1.0000x reference)
"""Trainium2 Bass kernel for nn_Attention_32762010534254.

Cross-attention: q(B,Nq,D) kv(B,Nkv,D) -> softmax((qWq)(kvWk)^T/sqrt(dh)) (kvWv) Wo + bo
B=2, Nq=512, Nkv=4096, D=1024, heads=16, dh=64.

Sharding (8 cores): core i handles batch b=i//4 and head group g=i%4
(4 heads = 2 pairs). Per-core device work:
  - inputs (kv, q) and projection weights uploaded as bf16 (halves DMA;
    matmuls stay 1 cycle/row on the PE, PSUM accumulation is f32)
  - Q projection then chunk-0 K/V projection streamed ko-major so the PE
    consumes DMA strips at arrival rate; later chunks' projections are
    interleaved with the previous chunk's attention, kv chunk DMAs issued
    two iterations ahead
  - S^T = K_h Q_h^T with keys on partitions (row-tiled head pairs, K=64)
  - exp via ACT with fused 1/8 scale + per-key mask bias -> bf16 probs
    (no max subtraction: scores are O(1) by construction, fp32 exp
    cannot overflow)
  - AV in O-layout: opsum[q, 65] += pt[:, qslice]^T @ V_aug, moving dim
    65 (vs 512 for O^T layout) -> half the AV PE time; the ones column
    of V_aug accumulates softmax sums per q on partitions. AV matmuls
    lag their exps by AV_LAG entries so the backlog fills the PE during
    the last chunk (no projection work left) and the final drain is
    head-ordered so heads 2/3 fill the pair-0 normalize window
  - normalize with per-partition reciprocal scalars (ACT/DVE split),
    PE-transpose X -> X^T via identity matmul, row-parallel Wo partial,
    one batched bf16 store per q-tile (f32 stores / per-512-col stores
    are DMA- resp. HWDGE-generation-bound in the tail)
Host: shards inputs (transposes q/kv once, bf16 cast), upcasts and sums
the 4 bf16 partials per batch in f32, +bo.

Self-contained: hardcodes all shapes; requires concourse + numpy + ml_dtypes.
"""

import os

import numpy as np
import ml_dtypes

import concourse.bass as bass  # noqa: F401  (bass types via bacc/tile)
import concourse.tile as tile
from concourse import bacc, mybir
from concourse import bass_utils

F32 = mybir.dt.float32
F32R = mybir.dt.float32r
BF16 = mybir.dt.bfloat16
EXP = mybir.ActivationFunctionType.Exp

B, NQ, NKV, D = 2, 512, 4096, 1024
HEADS, DH = 16, 64
SCALE = DH ** -0.5
N_CORES = 8
HPC = HEADS // (N_CORES // B)   # heads per core = 4
PAIRS = HPC // 2                # head pairs per core = 2
KO = D // 128                   # 8 contraction sub-chunks
# kv chunk sizes in 128-key groups: small first chunk so attention starts
# before the bulk of the prologue DMA lands; small last chunk to shorten
# the final ACT-bound exp wave
SIZES = [int(x) for x in os.environ.get("SIZES", "4,4,4,4,4,4,4,4").split(",")]
assert sum(SIZES) * 128 == NKV
NCHUNK = len(SIZES)
STARTS = [sum(SIZES[:i]) for i in range(NCHUNK)]
TOTAL_GROUPS = sum(SIZES)

KV_BUFS = int(os.environ.get("KV_BUFS", "3"))
AV_LAG = int(os.environ.get("AV_LAG", "88"))
AV_DRAIN = int(os.environ.get("AV_DRAIN", "20"))
PT_BUFS = int(os.environ.get("PT_BUFS", "118"))
V_BUFS = int(os.environ.get("V_BUFS", "7"))
PSS_BUFS = int(os.environ.get("PSS_BUFS", "2"))
_NC_CACHE = []


def _build_nc():
    nc = bacc.Bacc("TRN2", target_bir_lowering=False, debug=False,
                   num_devices=N_CORES)
    qT = nc.dram_tensor("qT", [D, NQ], BF16, kind="ExternalInput").ap()
    kvT = nc.dram_tensor("kvT", [D, NKV], BF16, kind="ExternalInput").ap()
    wq = nc.dram_tensor("wq", [D, HPC * DH], BF16, kind="ExternalInput").ap()
    wkv = nc.dram_tensor("wkv", [D, 2 * HPC * DH], BF16, kind="ExternalInput").ap()
    wo = nc.dram_tensor("wo", [HPC * DH, D], F32R, kind="ExternalInput").ap()
    bias = nc.dram_tensor("bias", [128, TOTAL_GROUPS], F32, kind="ExternalInput").ap()
    ident = nc.dram_tensor("ident", [128, 128], F32R, kind="ExternalInput").ap()
    # bf16 partial-output stores halve the tail DMA; host upcasts + sums the
    # 4 per-batch partials in f32, so the quantization hit is ~0.1% of out
    out = nc.dram_tensor("out", [NQ, D], BF16, kind="ExternalOutput").ap()
    dbg = None
    if os.environ.get("DEBUG_DUMP") == "1":
        dbg = nc.dram_tensor("dbg", [HPC, 128, 4 * (DH + 1)], F32,
                             kind="ExternalOutput").ap()

    qT_r = qT.rearrange("(ko p) n -> p ko n", p=128)
    kvT_r = kvT.rearrange("(ko p) n -> p ko n", p=128)
    wq_r = wq.rearrange("(ko p) m -> p ko m", p=128)
    wkv_r = wkv.rearrange("(ko p) (t m) -> p ko t m", p=128, t=2)
    wo_r = wo.rearrange("(ko p) n -> p ko n", p=128)

    with tile.TileContext(nc) as tc:
        with (
            tc.tile_pool(name="const", bufs=1) as cpool,
            tc.tile_pool(name="kv", bufs=KV_BUFS) as kv_pool,
            tc.tile_pool(name="kt", bufs=2) as kt_pool,
            tc.tile_pool(name="v", bufs=V_BUFS) as v_pool,
            tc.tile_pool(name="pt", bufs=PT_BUFS) as p_pool,
            tc.tile_pool(name="xp", bufs=4) as xp_pool,
            tc.tile_pool(name="ob", bufs=4) as o_pool,
            tc.tile_pool(name="psA", bufs=1, space="PSUM") as psA,
            tc.tile_pool(name="psV", bufs=1, space="PSUM") as psV,
            tc.tile_pool(name="psS", bufs=PSS_BUFS, space="PSUM") as psS,
            tc.tile_pool(name="psO", bufs=1, space="PSUM") as psO,
        ):
            wq_sb = cpool.tile([128, KO, HPC * DH], BF16, tag="wq")
            wkv_sb = cpool.tile([128, KO, 2, HPC * DH], BF16, tag="wkv")
            wo_sb = cpool.tile([128, PAIRS, D], F32R, tag="wo")
            qT_sb = cpool.tile([128, KO, NQ], BF16, tag="qT")
            bias_sb = cpool.tile([128, TOTAL_GROUPS], F32, tag="bias")
            ident_sb = cpool.tile([128, 128], F32R, tag="ident")
            qh_sb = cpool.tile([128, PAIRS, NQ], BF16, tag="qh")
            xT = [cpool.tile([128, NQ], F32R, tag=f"x{p}", name=f"xT{p}")
                  for p in range(PAIRS)]
            rsb = [cpool.tile([128, 4], F32, tag=f"r{h}", name=f"rsb{h}")
                   for h in range(HPC)]

            # warmup scratch: dummy matmuls placed into known prologue DMA
            # stalls keep the PE's p-state ramp alive (a gap drops the clock
            # to 1.2GHz for 3us of the following real matmuls)
            dsb = cpool.tile([128, 512], BF16, tag="dsb")
            nc.vector.memset(dsb[:], 0.0)
            warm_n = [0]

            def warm(n):
                for _ in range(n):
                    wps = psS.tile([128, 512], F32, tag="psS",
                                   name=f"warm{warm_n[0]}")
                    warm_n[0] += 1
                    nc.tensor.matmul(wps[:], dsb[0:128, 0:128], dsb[:, :],
                                     start=True, stop=True)

            # prologue DMAs, strip-granular and ordered exactly by first use:
            # Q projection streams first (per-ko, both pairs), then chunk-0
            # K projection ko-major, then V projection, then chunk 1
            kvc0 = kv_pool.tile([128, KO, SIZES[0] * 128], BF16, tag="kvc",
                                name="kvc0")
            kvc1 = kv_pool.tile([128, KO, SIZES[1] * 128], BF16, tag="kvc",
                                name="kvc1")
            nc.sync.dma_start(wq_sb[:, 0:4, :], wq_r[:, 0:4, :])
            nc.sync.dma_start(qT_sb[:, 0:2, :], qT_r[:, 0:2, :])
            nc.sync.dma_start(wq_sb[:, 4:KO, :], wq_r[:, 4:KO, :])
            nc.sync.dma_start(qT_sb[:, 2:4, :], qT_r[:, 2:4, :])
            nc.sync.dma_start(qT_sb[:, 4:KO, :], qT_r[:, 4:KO, :])
            nc.sync.dma_start(wkv_sb[:, 0:4, :, :], wkv_r[:, 0:4, :, :])
            nc.sync.dma_start(kvc0[:, 0:2, :], kvT_r[:, 0:2, 0:SIZES[0] * 128])
            nc.sync.dma_start(kvc0[:, 2:4, :], kvT_r[:, 2:4, 0:SIZES[0] * 128])
            nc.sync.dma_start(wkv_sb[:, 4:KO, :, :], wkv_r[:, 4:KO, :, :])
            nc.sync.dma_start(kvc0[:, 4:6, :], kvT_r[:, 4:6, 0:SIZES[0] * 128])
            nc.sync.dma_start(kvc0[:, 6:KO, :], kvT_r[:, 6:KO, 0:SIZES[0] * 128])
            nc.sync.dma_start(
                kvc1[:], kvT_r[:, :, STARTS[1] * 128:(STARTS[1] + SIZES[1]) * 128])
            nc.sync.dma_start(bias_sb[:], bias)
            nc.sync.dma_start(ident_sb[:], ident)

            def q_projection():
                # ko-major over both pairs so the PE consumes qT strips as
                # they stream in
                qp0 = psA.tile([128, NQ], F32, tag="psA", name="qp0")
                qp1 = psV.tile([128, NQ], F32, tag="psV", name="qp1")
                for ko in range(KO):
                    nc.tensor.matmul(qp0[:], wq_sb[:, ko, 0:128],
                                     qT_sb[:, ko, :], start=(ko == 0),
                                     stop=(ko == KO - 1))
                    nc.tensor.matmul(qp1[:], wq_sb[:, ko, 128:256],
                                     qT_sb[:, ko, :], start=(ko == 0),
                                     stop=(ko == KO - 1))
                nc.vector.tensor_copy(qh_sb[:, 0, :], qp0[:])
                nc.scalar.copy(qh_sb[:, 1, :], qp1[:])

            def proj_c0(kvc, ktc, vc):
                # chunk-0 projection is ko-major across 4 concurrent PSUM
                # accumulators (both K pairs + first two V subs) so the PE
                # consumes kv strips at streaming rate; V m2/m3 follow once
                # the whole chunk has landed
                kp0 = psA.tile([128, SIZES[0] * 128], F32, tag="psA", name="kp0_0")
                kp1 = psV.tile([128, SIZES[0] * 128], F32, tag="psV", name="kp0_1")
                n_ko_major = min(2, SIZES[0])
                vps = [psS.tile([128, HPC * DH], F32, tag="psS", name=f"vp0_{m}")
                       for m in range(n_ko_major)]
                for ko in range(KO):
                    first, last = ko == 0, ko == KO - 1
                    nc.tensor.matmul(kp0[:], wkv_sb[:, ko, 0, 0:128],
                                     kvc[:, ko, :], start=first, stop=last)
                    nc.tensor.matmul(kp1[:], wkv_sb[:, ko, 0, 128:256],
                                     kvc[:, ko, :], start=first, stop=last)
                    for m in range(n_ko_major):
                        nc.tensor.matmul(vps[m][:], kvc[:, ko, 128 * m:128 * (m + 1)],
                                         wkv_sb[:, ko, 1, :], start=first, stop=last)
                nc.vector.tensor_copy(ktc[:, 0, :], kp0[:])
                nc.scalar.copy(ktc[:, 1, :], kp1[:])
                for m in range(n_ko_major):
                    nc.vector.tensor_copy(
                        vc[:, m, :, 0:DH],
                        vps[m][:].rearrange("p (h d) -> p h d", h=HPC),
                    )
                for m in range(n_ko_major, SIZES[0]):
                    proj_v_sub(0, kvc, vc, m, pool=(psA, psV)[m % 2])

            # persistent O accumulators, one bank per head:
            # [128 q-partitions, 4 qtiles x (dh + ones-column sums)].
            # Zeroed explicitly: a start=True matmul resets the whole PSUM
            # bank, clobbering sibling qtile regions, so every AV matmul
            # accumulates (start=False) onto these zeros instead.
            opsum = [psO.tile([128, 4 * (DH + 1)], F32, tag=f"o{h}", name=f"opsum{h}")
                     for h in range(HPC)]
            for h in range(HPC):
                nc.vector.memset(opsum[h][:], 0.0)

            def load_chunk(c):
                if c == 0:
                    return kvc0
                if c == 1:
                    return kvc1
                kvc = kv_pool.tile([128, KO, SIZES[c] * 128], BF16, tag="kvc",
                                   name=f"kvc{c}")
                nc.sync.dma_start(
                    kvc[:],
                    kvT_r[:, :, STARTS[c] * 128:(STARTS[c] + SIZES[c]) * 128])
                return kvc

            def proj_k_pair(c, kvc, ktc, p):
                kp = psA.tile([128, SIZES[c] * 128], F32, tag="psA",
                              name=f"kp{c}_{p}")
                for ko in range(KO):
                    nc.tensor.matmul(
                        kp[:], wkv_sb[:, ko, 0, 128 * p:128 * (p + 1)],
                        kvc[:, ko, :], start=(ko == 0), stop=(ko == KO - 1),
                    )
                nc.vector.tensor_copy(ktc[:, p, :], kp[:])

            def proj_v_sub(c, kvc, vc, m, pool=None):
                pool = pool or psV
                vp = pool.tile([128, HPC * DH], F32, tag=pool.name, name=f"vp{c}_{m}")
                for ko in range(KO):
                    nc.tensor.matmul(
                        vp[:], kvc[:, ko, 128 * m:128 * (m + 1)],
                        wkv_sb[:, ko, 1, :], start=(ko == 0), stop=(ko == KO - 1),
                    )
                nc.vector.tensor_copy(
                    vc[:, m, :, 0:DH],
                    vp[:].rearrange("p (h d) -> p h d", h=HPC),
                )

            def alloc_proj_tiles(c):
                ktc = kt_pool.tile([128, PAIRS, SIZES[c] * 128], BF16, tag="ktc",
                                   name=f"ktc{c}")
                vc = v_pool.tile([128, SIZES[c], HPC, DH + 1], BF16, tag="vc",
                                 name=f"vc{c}")
                nc.vector.memset(vc[:, :, :, DH:DH + 1], 1.0)
                return ktc, vc

            def proj_pieces(c, kvc, ktc, vc):
                # closures, emitted spread across the previous chunk's slots
                ps = [lambda p=p: proj_k_pair(c, kvc, ktc, p) for p in range(PAIRS)]
                ps += [lambda m=m: proj_v_sub(c, kvc, vc, m)
                       for m in range(SIZES[c])]
                return ps

            av_pending = []

            def qk_exp_group(c, ktc, vc, s, mid_flush=None):
                g = STARTS[c] + s
                bias_ap = bias_sb[:, g:g + 1]
                for p in range(PAIRS):
                    sps = []
                    for half in range(2):  # row-tiled pair, K=64
                        lo, hi = 64 * half, 64 * (half + 1)
                        sp = psS.tile([128, NQ], F32, tag="psS", name=f"sp{c}_{s}_{p}_{half}")
                        nc.tensor.matmul(
                            sp[:], ktc[lo:hi, p, 128 * s:128 * (s + 1)],
                            qh_sb[lo:hi, p, :], start=True, stop=True,
                        )
                        sps.append(sp)
                    for half, sp in enumerate(sps):
                        h = 2 * p + half
                        pt = p_pool.tile([128, NQ], BF16, tag="pt", name=f"pt{c}_{s}_{p}_{half}")
                        nc.scalar.activation(
                            pt[:], sp[:], EXP, bias=bias_ap, scale=SCALE,
                        )
                        av_pending.append((c, s, h, vc, pt))
                    if p == 0 and mid_flush is not None:
                        # keep the PE fed while the second pair's QK matmuls
                        # wait on PSUM banks held by in-flight exps
                        flush_av(mid_flush)

            def emit_av(entry):
                c, s, h, vc, pt = entry
                for m in range(4):
                    nc.tensor.matmul(
                        opsum[h][:, 65 * m:65 * (m + 1)],
                        pt[:, 128 * m:128 * (m + 1)],
                        vc[:, s, h, :],
                        start=False,
                        stop=(STARTS[c] + s == TOTAL_GROUPS - 1),
                        skip_group_check=True,
                    )

            def flush_av(upto):
                # emit AV matmuls (O-layout, moving dim 65) for groups that
                # have lagged enough behind their exp
                while av_pending and len(av_pending) > upto:
                    emit_av(av_pending.pop(0))

            # prologue: Q projection (streams qT), then chunk-0 projection,
            # with warmup dummies covering the DMA-wait gaps
            kvc_cur = load_chunk(0)
            ktc_cur, vc_cur = alloc_proj_tiles(0)
            warm(int(os.environ.get("WARM0", "8")))
            q_projection()
            warm(int(os.environ.get("WARM1", "0")))
            proj_c0(kvc_cur, ktc_cur, vc_cur)
            warm(int(os.environ.get("WARM2", "0")))

            # steady state: attention(c) interleaved with projections(c+1)
            # spread across c's attention slots; AV lags behind its exp to
            # hide ACT->PE sem latency. kv chunk DMAs are issued two
            # iterations ahead of their projection. The second-to-last chunk
            # has spare slot time (small next chunk) and the last chunk has
            # none at all, so the AV backlog drains through their ACT-bound
            # windows on a stepped schedule.
            kvc_next = {0: kvc0, 1: kvc1}
            drain7 = int(os.environ.get("AV_DRAIN7", "0"))
            for c in range(NCHUNK):
                if c + 2 < NCHUNK:
                    kvc_next[c + 2] = load_chunk(c + 2)
                if c == 0:
                    nc.sync.dma_start(wo_sb[:], wo_r)
                pieces = []
                if c + 1 < NCHUNK:
                    kvc_nxt = kvc_next[c + 1]
                    ktc_nxt, vc_nxt = alloc_proj_tiles(c + 1)
                    pieces = proj_pieces(c + 1, kvc_nxt, ktc_nxt, vc_nxt)
                slots = SIZES[c]
                npieces = len(pieces)
                if slots == 4 and npieces == 6:
                    bounds = [0, 1, 2, 4, 6]  # K0 | K1 | V0 V1 | V2 V3
                else:
                    bounds = [npieces * s // slots for s in range(slots + 1)]
                for s in range(slots):
                    if c == NCHUNK - 1:
                        # final chunk: steep drain, mid-group flush keeps the
                        # PE fed while the last exps hold the score banks
                        target = max(0, AV_LAG - drain7 * 4
                                     - AV_DRAIN * (s + 1))
                        mid = AV_DRAIN // 2 if s < slots - 1 else 2
                        qk_exp_group(c, ktc_cur, vc_cur, s,
                                     mid_flush=target + mid)
                        flush_av(target)
                        continue
                    qk_exp_group(c, ktc_cur, vc_cur, s)
                    if c == NCHUNK - 2:
                        flush_av(max(0, AV_LAG - drain7 * (s + 1)))
                    else:
                        flush_av(AV_LAG)
                    for piece in pieces[bounds[s]:bounds[s + 1]]:
                        piece()
                if c + 1 < NCHUNK:
                    kvc_cur, ktc_cur, vc_cur = kvc_nxt, ktc_nxt, vc_nxt

            # head-ordered final drain: finish heads 0/1 first so their
            # normalize chain can start while heads 2/3's AV matmuls fill
            # the PE gaps between pair-0 transposes
            rest = sorted(av_pending, key=lambda e: e[2])
            av_pending.clear()
            lo_avs = [e for e in rest if e[2] < 2]
            hi_avs = [e for e in rest if e[2] >= 2]
            for e in lo_avs:
                emit_av(e)
            hi_batches = [hi_avs[i::4] for i in range(4)]

            # normalize: reciprocal of the ones-column sums (per-partition),
            # X = O * (1/sum) via per-partition scalar on ACT/DVE, then
            # PE-transpose X -> X^T via identity matmul
            if dbg is not None:
                for batch in hi_batches:
                    for e in batch:
                        emit_av(e)
                hi_batches = [[], [], [], []]
                for h in range(HPC):
                    dsb = o_pool.tile([128, 4 * (DH + 1)], F32, tag="dsb",
                                      name=f"dsb{h}")
                    nc.vector.tensor_copy(dsb[:], opsum[h][:])
                    nc.sync.dma_start(dbg[h], dsb[:])

            wo_pools = [psA, psV]

            def emit_wo(m):
                # one batched [128, 1024] bf16 store per q-tile: small stores
                # are HWDGE-generation-bound (632ns each), batching amortizes.
                # The final q-tile stores per-half so the end-of-kernel drain
                # only waits on a 512-column transfer.
                osb = o_pool.tile([128, D], BF16, tag="osb", name=f"osb{m}")
                for n in range(D // 512):
                    pool_w = wo_pools[n]
                    wp = pool_w.tile([128, 512], F32, tag=pool_w.name, name=f"wp{m}_{n}")
                    for p in range(PAIRS):
                        nc.tensor.matmul(
                            wp[:], xT[p][:, 128 * m:128 * (m + 1)],
                            wo_sb[:, p, 512 * n:512 * (n + 1)],
                            start=(p == 0), stop=(p == PAIRS - 1),
                        )
                    if n % 2 == 0:
                        nc.vector.tensor_copy(osb[:, 0:512], wp[:])
                    else:
                        nc.scalar.copy(osb[:, 512:D], wp[:])
                    if m == 3:
                        nc.sync.dma_start(
                            out[128 * m:128 * (m + 1), 512 * n:512 * (n + 1)],
                            osb[:, 512 * n:512 * (n + 1)])
                if m < 3:
                    nc.sync.dma_start(out[128 * m:128 * (m + 1), :], osb[:])

            # pair-major tail: pair 0's heads finish their exps/AV ~1.2us
            # before pair 1's, so normalize+transpose pair 0 while pair 1's
            # exps drain; Wo for qtile m follows pair 1's transpose of m.
            # TAIL_MODE=m emits qtile-major instead (AV drain upfront).
            def norm_unit(p, m):
                hA, hB = 2 * p, 2 * p + 1
                xp_sb = xp_pool.tile([128, 128], F32R, tag="xp", name=f"xp{m}_{p}")
                nc.scalar.mul(
                    xp_sb[:, 0:DH], opsum[hA][:, 65 * m:65 * m + DH],
                    rsb[hA][:, m:m + 1],
                )
                nc.vector.tensor_scalar_mul(
                    xp_sb[:, DH:128], opsum[hB][:, 65 * m:65 * m + DH],
                    rsb[hB][:, m:m + 1],
                )
                xtp = psS.tile([128, 128], F32R, tag="psS", name=f"xtp{m}_{p}")
                nc.tensor.transpose(xtp[:], xp_sb[:], ident_sb[:])
                if m % 2 == 0:
                    nc.vector.tensor_copy(xT[p][:, 128 * m:128 * (m + 1)], xtp[:])
                else:
                    nc.scalar.copy(xT[p][:, 128 * m:128 * (m + 1)], xtp[:])

            def recips(p):
                for hh in (2 * p, 2 * p + 1):
                    sums = opsum[hh].rearrange("p (m e) -> p m e", e=DH + 1)[:, :, DH]
                    nc.vector.reciprocal(rsb[hh][:], sums)

            if os.environ.get("TAIL_MODE", "m") == "m":
                for batch in hi_batches:
                    for e in batch:
                        emit_av(e)
                recips(0)
                recips(1)
                for m in range(4):
                    norm_unit(0, m)
                    norm_unit(1, m)
                    emit_wo(m)
            else:
                for p in range(PAIRS):
                    recips(p)
                    for m in range(4):
                        norm_unit(p, m)
                        if p == 0:
                            for e in hi_batches[m]:
                                emit_av(e)
                        else:
                            emit_wo(m)

    nc.compile()
    return nc


def _get_nc():
    if not _NC_CACHE:
        _NC_CACHE.append(_build_nc())
    return _NC_CACHE[0]


LAST_RESULTS = None


def kernel(q, kv, mask, Wq, Wkv, Wo, bo):
    global LAST_RESULTS
    q = np.asarray(q, dtype=np.float32)
    kv = np.asarray(kv, dtype=np.float32)
    mask = np.asarray(mask)
    Wq = np.asarray(Wq, dtype=np.float32)
    Wkv = np.asarray(Wkv, dtype=np.float32)
    Wo = np.asarray(Wo, dtype=np.float32)
    bo = np.asarray(bo, dtype=np.float32)

    bf16 = ml_dtypes.bfloat16
    inner = HEADS * DH
    qT = [np.ascontiguousarray(q[b].T.astype(bf16)) for b in range(B)]
    kvT = [np.ascontiguousarray(kv[b].T.astype(bf16)) for b in range(B)]
    ident = np.eye(128, dtype=np.float32)
    bias = []
    for b in range(B):
        bb = np.where(mask[b], 0.0, -30000.0).astype(np.float32)
        bias.append(np.ascontiguousarray(bb.reshape(TOTAL_GROUPS, 128).T))

    in_maps = []
    for i in range(N_CORES):
        b, g = divmod(i, N_CORES // B)
        cs = slice(HPC * DH * g, HPC * DH * (g + 1))
        in_maps.append({
            "qT": qT[b],
            "kvT": kvT[b],
            "wq": np.ascontiguousarray(Wq[:, cs].astype(bf16)),
            "wkv": np.ascontiguousarray(np.concatenate(
                [Wkv[:, cs], Wkv[:, inner:][:, cs]], axis=1).astype(bf16)),
            "wo": np.ascontiguousarray(Wo[cs, :]),
            "bias": bias[b],
            "ident": ident,
        })

    nc = _get_nc()
    res = bass_utils.run_bass_kernel_spmd(
        nc, in_maps, core_ids=list(range(N_CORES)))
    LAST_RESULTS = res

    gpb = N_CORES // B
    out = np.zeros((B, NQ, D), np.float32)
    for b in range(B):
        acc = res.results[b * gpb]["out"].astype(np.float32).copy()
        for g in range(1, gpb):
            acc += res.results[b * gpb + g]["out"].astype(np.float32)
        out[b] = acc + bo[None, :]
    return out


# revision 71
# speedup vs baseline: 1.0044x; 1.0044x over previous
"""Trainium2 Bass kernel for nn_Attention_32762010534254.

Cross-attention: q(B,Nq,D) kv(B,Nkv,D) -> softmax((qWq)(kvWk)^T/sqrt(dh)) (kvWv) Wo + bo
B=2, Nq=512, Nkv=4096, D=1024, heads=16, dh=64.

Sharding (8 cores): core i handles batch b=i//4 and head group g=i%4
(4 heads = 2 pairs). Per-core device work:
  - inputs (kv, q) and projection weights uploaded as bf16 (halves DMA;
    matmuls stay 1 cycle/row on the PE, PSUM accumulation is f32)
  - Q projection then chunk-0 K/V projection streamed ko-major so the PE
    consumes DMA strips at arrival rate; later chunks' projections are
    interleaved with the previous chunk's attention, kv chunk DMAs issued
    two iterations ahead
  - S^T = K_h Q_h^T with keys on partitions (row-tiled head pairs, K=64)
  - exp via ACT with fused 1/8 scale + per-key mask bias -> bf16 probs
    (no max subtraction: scores are O(1) by construction, fp32 exp
    cannot overflow)
  - AV in O-layout: opsum[q, 65] += pt[:, qslice]^T @ V_aug, moving dim
    65 (vs 512 for O^T layout) -> half the AV PE time; the ones column
    of V_aug accumulates softmax sums per q on partitions. AV matmuls
    lag their exps by AV_LAG entries so the backlog fills the PE during
    the last chunk (no projection work left) and the final drain is
    head-ordered so heads 2/3 fill the pair-0 normalize window
  - normalize with per-partition reciprocal scalars (ACT/DVE split),
    PE-transpose X -> X^T via identity matmul, row-parallel Wo partial,
    one batched bf16 store per q-tile (f32 stores / per-512-col stores
    are DMA- resp. HWDGE-generation-bound in the tail)
Host: shards inputs (transposes q/kv once, bf16 cast), upcasts and sums
the 4 bf16 partials per batch in f32, +bo.

Self-contained: hardcodes all shapes; requires concourse + numpy + ml_dtypes.
"""

import os

import numpy as np
import ml_dtypes

import concourse.bass as bass  # noqa: F401  (bass types via bacc/tile)
import concourse.tile as tile
from concourse import bacc, mybir
from concourse import bass_utils

F32 = mybir.dt.float32
F32R = mybir.dt.float32r
BF16 = mybir.dt.bfloat16
EXP = mybir.ActivationFunctionType.Exp

B, NQ, NKV, D = 2, 512, 4096, 1024
HEADS, DH = 16, 64
SCALE = DH ** -0.5
N_CORES = 8
HPC = HEADS // (N_CORES // B)   # heads per core = 4
PAIRS = HPC // 2                # head pairs per core = 2
KO = D // 128                   # 8 contraction sub-chunks
# kv chunk sizes in 128-key groups: small first chunk so attention starts
# before the bulk of the prologue DMA lands; small last chunk to shorten
# the final ACT-bound exp wave
SIZES = [int(x) for x in os.environ.get("SIZES", "4,4,4,4,4,4,4,4").split(",")]
assert sum(SIZES) * 128 == NKV
NCHUNK = len(SIZES)
STARTS = [sum(SIZES[:i]) for i in range(NCHUNK)]
TOTAL_GROUPS = sum(SIZES)

KV_BUFS = int(os.environ.get("KV_BUFS", "3"))
AV_LAG = int(os.environ.get("AV_LAG", "88"))
AV_DRAIN = int(os.environ.get("AV_DRAIN", "20"))
PT_BUFS = int(os.environ.get("PT_BUFS", "118"))
V_BUFS = int(os.environ.get("V_BUFS", "7"))
PSS_BUFS = int(os.environ.get("PSS_BUFS", "2"))
_NC_CACHE = []


def _build_nc():
    nc = bacc.Bacc("TRN2", target_bir_lowering=False, debug=False,
                   num_devices=N_CORES)
    qT = nc.dram_tensor("qT", [D, NQ], BF16, kind="ExternalInput").ap()
    kvT = nc.dram_tensor("kvT", [D, NKV], BF16, kind="ExternalInput").ap()
    wq = nc.dram_tensor("wq", [D, HPC * DH], BF16, kind="ExternalInput").ap()
    wkv = nc.dram_tensor("wkv", [D, 2 * HPC * DH], BF16, kind="ExternalInput").ap()
    wo = nc.dram_tensor("wo", [HPC * DH, D], F32R, kind="ExternalInput").ap()
    bias = nc.dram_tensor("bias", [128, TOTAL_GROUPS], F32, kind="ExternalInput").ap()
    ident = nc.dram_tensor("ident", [128, 128], F32R, kind="ExternalInput").ap()
    # bf16 partial-output stores halve the tail DMA; host upcasts + sums the
    # 4 per-batch partials in f32, so the quantization hit is ~0.1% of out
    out = nc.dram_tensor("out", [NQ, D], BF16, kind="ExternalOutput").ap()
    dbg = None
    if os.environ.get("DEBUG_DUMP") == "1":
        dbg = nc.dram_tensor("dbg", [HPC, 128, 4 * (DH + 1)], F32,
                             kind="ExternalOutput").ap()

    qT_r = qT.rearrange("(ko p) n -> p ko n", p=128)
    kvT_r = kvT.rearrange("(ko p) n -> p ko n", p=128)
    wq_r = wq.rearrange("(ko p) m -> p ko m", p=128)
    wkv_r = wkv.rearrange("(ko p) (t m) -> p ko t m", p=128, t=2)
    wo_r = wo.rearrange("(ko p) n -> p ko n", p=128)

    with tile.TileContext(nc) as tc:
        with (
            tc.tile_pool(name="const", bufs=1) as cpool,
            tc.tile_pool(name="kv", bufs=KV_BUFS) as kv_pool,
            tc.tile_pool(name="kt", bufs=2) as kt_pool,
            tc.tile_pool(name="v", bufs=V_BUFS) as v_pool,
            tc.tile_pool(name="pt", bufs=PT_BUFS) as p_pool,
            tc.tile_pool(name="xp", bufs=4) as xp_pool,
            tc.tile_pool(name="ob", bufs=4) as o_pool,
            tc.tile_pool(name="psA", bufs=1, space="PSUM") as psA,
            tc.tile_pool(name="psV", bufs=1, space="PSUM") as psV,
            tc.tile_pool(name="psS", bufs=PSS_BUFS, space="PSUM") as psS,
            tc.tile_pool(name="psO", bufs=1, space="PSUM") as psO,
        ):
            wq_sb = cpool.tile([128, KO, HPC * DH], BF16, tag="wq")
            wkv_sb = cpool.tile([128, KO, 2, HPC * DH], BF16, tag="wkv")
            wo_sb = cpool.tile([128, PAIRS, D], F32R, tag="wo")
            qT_sb = cpool.tile([128, KO, NQ], BF16, tag="qT")
            bias_sb = cpool.tile([128, TOTAL_GROUPS], F32, tag="bias")
            ident_sb = cpool.tile([128, 128], F32R, tag="ident")
            qh_sb = cpool.tile([128, PAIRS, NQ], BF16, tag="qh")
            xT = [cpool.tile([128, NQ], F32R, tag=f"x{p}", name=f"xT{p}")
                  for p in range(PAIRS)]
            rsb = [cpool.tile([128, 4], F32, tag=f"r{h}", name=f"rsb{h}")
                   for h in range(HPC)]

            # warmup scratch: dummy matmuls placed into known prologue DMA
            # stalls keep the PE's p-state ramp alive (a gap drops the clock
            # to 1.2GHz for 3us of the following real matmuls)
            dsb = cpool.tile([128, 512], BF16, tag="dsb")
            nc.vector.memset(dsb[:], 0.0)
            warm_n = [0]

            def warm(n):
                for _ in range(n):
                    wps = psS.tile([128, 512], F32, tag="psS",
                                   name=f"warm{warm_n[0]}")
                    warm_n[0] += 1
                    nc.tensor.matmul(wps[:], dsb[0:128, 0:128], dsb[:, :],
                                     start=True, stop=True)

            # prologue DMAs, strip-granular and ordered exactly by first use:
            # Q projection streams first (per-ko, both pairs), then chunk-0
            # K projection ko-major, then V projection, then chunk 1
            kvc0 = kv_pool.tile([128, KO, SIZES[0] * 128], BF16, tag="kvc",
                                name="kvc0")
            kvc1 = kv_pool.tile([128, KO, SIZES[1] * 128], BF16, tag="kvc",
                                name="kvc1")
            nc.sync.dma_start(wq_sb[:, 0:4, :], wq_r[:, 0:4, :])
            nc.sync.dma_start(qT_sb[:, 0:2, :], qT_r[:, 0:2, :])
            nc.sync.dma_start(wq_sb[:, 4:KO, :], wq_r[:, 4:KO, :])
            nc.sync.dma_start(qT_sb[:, 2:4, :], qT_r[:, 2:4, :])
            nc.sync.dma_start(qT_sb[:, 4:KO, :], qT_r[:, 4:KO, :])
            nc.sync.dma_start(wkv_sb[:, 0:4, :, :], wkv_r[:, 0:4, :, :])
            nc.sync.dma_start(kvc0[:, 0:2, :], kvT_r[:, 0:2, 0:SIZES[0] * 128])
            nc.sync.dma_start(kvc0[:, 2:4, :], kvT_r[:, 2:4, 0:SIZES[0] * 128])
            nc.sync.dma_start(wkv_sb[:, 4:KO, :, :], wkv_r[:, 4:KO, :, :])
            nc.sync.dma_start(kvc0[:, 4:6, :], kvT_r[:, 4:6, 0:SIZES[0] * 128])
            nc.sync.dma_start(kvc0[:, 6:KO, :], kvT_r[:, 6:KO, 0:SIZES[0] * 128])
            nc.sync.dma_start(
                kvc1[:], kvT_r[:, :, STARTS[1] * 128:(STARTS[1] + SIZES[1]) * 128])
            nc.sync.dma_start(bias_sb[:], bias)
            nc.sync.dma_start(ident_sb[:], ident)

            def q_projection():
                # ko-major over both pairs so the PE consumes qT strips as
                # they stream in
                qp0 = psA.tile([128, NQ], F32, tag="psA", name="qp0")
                qp1 = psV.tile([128, NQ], F32, tag="psV", name="qp1")
                for ko in range(KO):
                    nc.tensor.matmul(qp0[:], wq_sb[:, ko, 0:128],
                                     qT_sb[:, ko, :], start=(ko == 0),
                                     stop=(ko == KO - 1))
                    nc.tensor.matmul(qp1[:], wq_sb[:, ko, 128:256],
                                     qT_sb[:, ko, :], start=(ko == 0),
                                     stop=(ko == KO - 1))
                nc.vector.tensor_copy(qh_sb[:, 0, :], qp0[:])
                nc.scalar.copy(qh_sb[:, 1, :], qp1[:])

            def proj_c0(kvc, ktc, vc):
                # chunk-0 projection is ko-major across 4 concurrent PSUM
                # accumulators (both K pairs + first two V subs) so the PE
                # consumes kv strips at streaming rate; V m2/m3 follow once
                # the whole chunk has landed
                kp0 = psA.tile([128, SIZES[0] * 128], F32, tag="psA", name="kp0_0")
                kp1 = psV.tile([128, SIZES[0] * 128], F32, tag="psV", name="kp0_1")
                n_ko_major = min(2, SIZES[0])
                vps = [psS.tile([128, HPC * DH], F32, tag="psS", name=f"vp0_{m}")
                       for m in range(n_ko_major)]
                for ko in range(KO):
                    first, last = ko == 0, ko == KO - 1
                    nc.tensor.matmul(kp0[:], wkv_sb[:, ko, 0, 0:128],
                                     kvc[:, ko, :], start=first, stop=last)
                    nc.tensor.matmul(kp1[:], wkv_sb[:, ko, 0, 128:256],
                                     kvc[:, ko, :], start=first, stop=last)
                    for m in range(n_ko_major):
                        nc.tensor.matmul(vps[m][:], kvc[:, ko, 128 * m:128 * (m + 1)],
                                         wkv_sb[:, ko, 1, :], start=first, stop=last)
                nc.vector.tensor_copy(ktc[:, 0, :], kp0[:])
                nc.scalar.copy(ktc[:, 1, :], kp1[:])
                for m in range(n_ko_major):
                    nc.vector.tensor_copy(
                        vc[:, m, :, 0:DH],
                        vps[m][:].rearrange("p (h d) -> p h d", h=HPC),
                    )
                for m in range(n_ko_major, SIZES[0]):
                    proj_v_sub(0, kvc, vc, m, pool=(psA, psV)[m % 2])

            # persistent O accumulators, one bank per head:
            # [128 q-partitions, 4 qtiles x (dh + ones-column sums)].
            # Zeroed explicitly: a start=True matmul resets the whole PSUM
            # bank, clobbering sibling qtile regions, so every AV matmul
            # accumulates (start=False) onto these zeros instead.
            opsum = [psO.tile([128, 4 * (DH + 1)], F32, tag=f"o{h}", name=f"opsum{h}")
                     for h in range(HPC)]
            for h in range(HPC):
                nc.vector.memset(opsum[h][:], 0.0)

            def load_chunk(c):
                if c == 0:
                    return kvc0
                if c == 1:
                    return kvc1
                kvc = kv_pool.tile([128, KO, SIZES[c] * 128], BF16, tag="kvc",
                                   name=f"kvc{c}")
                nc.sync.dma_start(
                    kvc[:],
                    kvT_r[:, :, STARTS[c] * 128:(STARTS[c] + SIZES[c]) * 128])
                return kvc

            def proj_k_pair(c, kvc, ktc, p):
                kp = psA.tile([128, SIZES[c] * 128], F32, tag="psA",
                              name=f"kp{c}_{p}")
                for ko in range(KO):
                    nc.tensor.matmul(
                        kp[:], wkv_sb[:, ko, 0, 128 * p:128 * (p + 1)],
                        kvc[:, ko, :], start=(ko == 0), stop=(ko == KO - 1),
                    )
                nc.vector.tensor_copy(ktc[:, p, :], kp[:])

            def proj_v_sub(c, kvc, vc, m, pool=None):
                pool = pool or psV
                vp = pool.tile([128, HPC * DH], F32, tag=pool.name, name=f"vp{c}_{m}")
                for ko in range(KO):
                    nc.tensor.matmul(
                        vp[:], kvc[:, ko, 128 * m:128 * (m + 1)],
                        wkv_sb[:, ko, 1, :], start=(ko == 0), stop=(ko == KO - 1),
                    )
                nc.vector.tensor_copy(
                    vc[:, m, :, 0:DH],
                    vp[:].rearrange("p (h d) -> p h d", h=HPC),
                )

            def alloc_proj_tiles(c):
                ktc = kt_pool.tile([128, PAIRS, SIZES[c] * 128], BF16, tag="ktc",
                                   name=f"ktc{c}")
                vc = v_pool.tile([128, SIZES[c], HPC, DH + 1], BF16, tag="vc",
                                 name=f"vc{c}")
                nc.vector.memset(vc[:, :, :, DH:DH + 1], 1.0)
                return ktc, vc

            def proj_pieces(c, kvc, ktc, vc):
                # closures, emitted spread across the previous chunk's slots
                ps = [lambda p=p: proj_k_pair(c, kvc, ktc, p) for p in range(PAIRS)]
                ps += [lambda m=m: proj_v_sub(c, kvc, vc, m)
                       for m in range(SIZES[c])]
                return ps

            av_pending = []

            def qk_exp_group(c, ktc, vc, s, mid_flush=None):
                g = STARTS[c] + s
                bias_ap = bias_sb[:, g:g + 1]
                for p in range(PAIRS):
                    sps = []
                    for half in range(2):  # row-tiled pair, K=64
                        lo, hi = 64 * half, 64 * (half + 1)
                        sp = psS.tile([128, NQ], F32, tag="psS", name=f"sp{c}_{s}_{p}_{half}")
                        nc.tensor.matmul(
                            sp[:], ktc[lo:hi, p, 128 * s:128 * (s + 1)],
                            qh_sb[lo:hi, p, :], start=True, stop=True,
                        )
                        sps.append(sp)
                    for half, sp in enumerate(sps):
                        h = 2 * p + half
                        pt = p_pool.tile([128, NQ], BF16, tag="pt", name=f"pt{c}_{s}_{p}_{half}")
                        nc.scalar.activation(
                            pt[:], sp[:], EXP, bias=bias_ap, scale=SCALE,
                        )
                        av_pending.append((c, s, h, vc, pt))
                    if p == 0 and mid_flush is not None:
                        # keep the PE fed while the second pair's QK matmuls
                        # wait on PSUM banks held by in-flight exps
                        flush_av(mid_flush)

            def emit_av(entry):
                c, s, h, vc, pt = entry
                for m in range(4):
                    nc.tensor.matmul(
                        opsum[h][:, 65 * m:65 * (m + 1)],
                        pt[:, 128 * m:128 * (m + 1)],
                        vc[:, s, h, :],
                        start=False,
                        stop=(STARTS[c] + s == TOTAL_GROUPS - 1),
                        skip_group_check=True,
                    )

            def flush_av(upto):
                # emit AV matmuls (O-layout, moving dim 65) for groups that
                # have lagged enough behind their exp
                while av_pending and len(av_pending) > upto:
                    emit_av(av_pending.pop(0))

            # prologue: Q projection (streams qT), then chunk-0 projection,
            # with warmup dummies covering the DMA-wait gaps
            kvc_cur = load_chunk(0)
            ktc_cur, vc_cur = alloc_proj_tiles(0)
            warm(int(os.environ.get("WARM0", "8")))
            q_projection()
            warm(int(os.environ.get("WARM1", "0")))
            proj_c0(kvc_cur, ktc_cur, vc_cur)
            warm(int(os.environ.get("WARM2", "0")))

            # steady state: attention(c) interleaved with projections(c+1)
            # spread across c's attention slots; AV lags behind its exp to
            # hide ACT->PE sem latency. kv chunk DMAs are issued two
            # iterations ahead of their projection. The second-to-last chunk
            # has spare slot time (small next chunk) and the last chunk has
            # none at all, so the AV backlog drains through their ACT-bound
            # windows on a stepped schedule.
            kvc_next = {0: kvc0, 1: kvc1}
            drain7 = int(os.environ.get("AV_DRAIN7", "0"))
            for c in range(NCHUNK):
                if c + 2 < NCHUNK:
                    kvc_next[c + 2] = load_chunk(c + 2)
                if c == 0:
                    nc.sync.dma_start(wo_sb[:], wo_r)
                pieces = []
                if c + 1 < NCHUNK:
                    kvc_nxt = kvc_next[c + 1]
                    ktc_nxt, vc_nxt = alloc_proj_tiles(c + 1)
                    pieces = proj_pieces(c + 1, kvc_nxt, ktc_nxt, vc_nxt)
                slots = SIZES[c]
                npieces = len(pieces)
                if slots == 4 and npieces == 6:
                    bounds = [0, 1, 2, 4, 6]  # K0 | K1 | V0 V1 | V2 V3
                else:
                    bounds = [npieces * s // slots for s in range(slots + 1)]
                for s in range(slots):
                    if c == NCHUNK - 1:
                        # final chunk: steep drain, mid-group flush keeps the
                        # PE fed while the last exps hold the score banks
                        target = max(0, AV_LAG - drain7 * 4
                                     - AV_DRAIN * (s + 1))
                        mid = AV_DRAIN // 2 if s < slots - 1 else 2
                        qk_exp_group(c, ktc_cur, vc_cur, s,
                                     mid_flush=target + mid)
                        flush_av(target)
                        continue
                    qk_exp_group(c, ktc_cur, vc_cur, s)
                    if c == NCHUNK - 2:
                        flush_av(max(0, AV_LAG - drain7 * (s + 1)))
                    else:
                        flush_av(AV_LAG)
                    for piece in pieces[bounds[s]:bounds[s + 1]]:
                        piece()
                if c + 1 < NCHUNK:
                    kvc_cur, ktc_cur, vc_cur = kvc_nxt, ktc_nxt, vc_nxt

            # head-ordered final drain: finish heads 0/1 first so their
            # normalize chain can start while heads 2/3's AV matmuls fill
            # the PE gaps between pair-0 transposes
            rest = sorted(av_pending, key=lambda e: e[2])
            av_pending.clear()
            lo_avs = [e for e in rest if e[2] < 2]
            hi_avs = [e for e in rest if e[2] >= 2]
            for e in lo_avs:
                emit_av(e)
            hi_batches = [hi_avs[i::4] for i in range(4)]

            # normalize: reciprocal of the ones-column sums (per-partition),
            # X = O * (1/sum) via per-partition scalar on ACT/DVE, then
            # PE-transpose X -> X^T via identity matmul
            if dbg is not None:
                for batch in hi_batches:
                    for e in batch:
                        emit_av(e)
                hi_batches = [[], [], [], []]
                for h in range(HPC):
                    dsb = o_pool.tile([128, 4 * (DH + 1)], F32, tag="dsb",
                                      name=f"dsb{h}")
                    nc.vector.tensor_copy(dsb[:], opsum[h][:])
                    nc.sync.dma_start(dbg[h], dsb[:])

            wo_pools = [psA, psV]

            def emit_wo(m):
                # one batched [128, 1024] bf16 store per q-tile: small stores
                # are HWDGE-generation-bound (632ns each), batching amortizes.
                # The final q-tile stores per-half so the end-of-kernel drain
                # only waits on a 512-column transfer.
                osb = o_pool.tile([128, D], BF16, tag="osb", name=f"osb{m}")
                for n in range(D // 512):
                    pool_w = wo_pools[n]
                    wp = pool_w.tile([128, 512], F32, tag=pool_w.name, name=f"wp{m}_{n}")
                    for p in range(PAIRS):
                        nc.tensor.matmul(
                            wp[:], xT[p][:, 128 * m:128 * (m + 1)],
                            wo_sb[:, p, 512 * n:512 * (n + 1)],
                            start=(p == 0), stop=(p == PAIRS - 1),
                        )
                    if n % 2 == 0:
                        nc.vector.tensor_copy(osb[:, 0:512], wp[:])
                    else:
                        nc.scalar.copy(osb[:, 512:D], wp[:])
                    if m >= 2:
                        nc.sync.dma_start(
                            out[128 * m:128 * (m + 1), 512 * n:512 * (n + 1)],
                            osb[:, 512 * n:512 * (n + 1)])
                if m < 2:
                    nc.sync.dma_start(out[128 * m:128 * (m + 1), :], osb[:])

            # pair-major tail: pair 0's heads finish their exps/AV ~1.2us
            # before pair 1's, so normalize+transpose pair 0 while pair 1's
            # exps drain; Wo for qtile m follows pair 1's transpose of m.
            # TAIL_MODE=m emits qtile-major instead (AV drain upfront).
            def norm_unit(p, m):
                hA, hB = 2 * p, 2 * p + 1
                xp_sb = xp_pool.tile([128, 128], F32R, tag="xp", name=f"xp{m}_{p}")
                nc.scalar.mul(
                    xp_sb[:, 0:DH], opsum[hA][:, 65 * m:65 * m + DH],
                    rsb[hA][:, m:m + 1],
                )
                nc.vector.tensor_scalar_mul(
                    xp_sb[:, DH:128], opsum[hB][:, 65 * m:65 * m + DH],
                    rsb[hB][:, m:m + 1],
                )
                xtp = psS.tile([128, 128], F32R, tag="psS", name=f"xtp{m}_{p}")
                nc.tensor.transpose(xtp[:], xp_sb[:], ident_sb[:])
                if m % 2 == 0:
                    nc.vector.tensor_copy(xT[p][:, 128 * m:128 * (m + 1)], xtp[:])
                else:
                    nc.scalar.copy(xT[p][:, 128 * m:128 * (m + 1)], xtp[:])

            def recips(p):
                for hh in (2 * p, 2 * p + 1):
                    sums = opsum[hh].rearrange("p (m e) -> p m e", e=DH + 1)[:, :, DH]
                    nc.vector.reciprocal(rsb[hh][:], sums)

            if os.environ.get("TAIL_MODE", "m") == "m":
                for batch in hi_batches:
                    for e in batch:
                        emit_av(e)
                recips(0)
                recips(1)
                for m in range(4):
                    norm_unit(0, m)
                    norm_unit(1, m)
                    emit_wo(m)
            else:
                for p in range(PAIRS):
                    recips(p)
                    for m in range(4):
                        norm_unit(p, m)
                        if p == 0:
                            for e in hi_batches[m]:
                                emit_av(e)
                        else:
                            emit_wo(m)

    nc.compile()
    return nc


def _get_nc():
    if not _NC_CACHE:
        _NC_CACHE.append(_build_nc())
    return _NC_CACHE[0]


LAST_RESULTS = None


def kernel(q, kv, mask, Wq, Wkv, Wo, bo):
    global LAST_RESULTS
    q = np.asarray(q, dtype=np.float32)
    kv = np.asarray(kv, dtype=np.float32)
    mask = np.asarray(mask)
    Wq = np.asarray(Wq, dtype=np.float32)
    Wkv = np.asarray(Wkv, dtype=np.float32)
    Wo = np.asarray(Wo, dtype=np.float32)
    bo = np.asarray(bo, dtype=np.float32)

    bf16 = ml_dtypes.bfloat16
    inner = HEADS * DH
    qT = [np.ascontiguousarray(q[b].T.astype(bf16)) for b in range(B)]
    kvT = [np.ascontiguousarray(kv[b].T.astype(bf16)) for b in range(B)]
    ident = np.eye(128, dtype=np.float32)
    bias = []
    for b in range(B):
        bb = np.where(mask[b], 0.0, -30000.0).astype(np.float32)
        bias.append(np.ascontiguousarray(bb.reshape(TOTAL_GROUPS, 128).T))

    in_maps = []
    for i in range(N_CORES):
        b, g = divmod(i, N_CORES // B)
        cs = slice(HPC * DH * g, HPC * DH * (g + 1))
        in_maps.append({
            "qT": qT[b],
            "kvT": kvT[b],
            "wq": np.ascontiguousarray(Wq[:, cs].astype(bf16)),
            "wkv": np.ascontiguousarray(np.concatenate(
                [Wkv[:, cs], Wkv[:, inner:][:, cs]], axis=1).astype(bf16)),
            "wo": np.ascontiguousarray(Wo[cs, :]),
            "bias": bias[b],
            "ident": ident,
        })

    nc = _get_nc()
    res = bass_utils.run_bass_kernel_spmd(
        nc, in_maps, core_ids=list(range(N_CORES)))
    LAST_RESULTS = res

    gpb = N_CORES // B
    out = np.zeros((B, NQ, D), np.float32)
    for b in range(B):
        acc = res.results[b * gpb]["out"].astype(np.float32).copy()
        for g in range(1, gpb):
            acc += res.results[b * gpb + g]["out"].astype(np.float32)
        out[b] = acc + bo[None, :]
    return out


# revision 72
# speedup vs baseline: 1.0052x; 1.0008x over previous
"""Trainium2 Bass kernel for nn_Attention_32762010534254.

Cross-attention: q(B,Nq,D) kv(B,Nkv,D) -> softmax((qWq)(kvWk)^T/sqrt(dh)) (kvWv) Wo + bo
B=2, Nq=512, Nkv=4096, D=1024, heads=16, dh=64.

Sharding (8 cores): core i handles batch b=i//4 and head group g=i%4
(4 heads = 2 pairs). Per-core device work:
  - inputs (kv, q) and projection weights uploaded as bf16 (halves DMA;
    matmuls stay 1 cycle/row on the PE, PSUM accumulation is f32)
  - Q projection then chunk-0 K/V projection streamed ko-major so the PE
    consumes DMA strips at arrival rate; later chunks' projections are
    interleaved with the previous chunk's attention, kv chunk DMAs issued
    two iterations ahead
  - S^T = K_h Q_h^T with keys on partitions (row-tiled head pairs, K=64)
  - exp via ACT with fused 1/8 scale + per-key mask bias -> bf16 probs
    (no max subtraction: scores are O(1) by construction, fp32 exp
    cannot overflow)
  - AV in O-layout: opsum[q, 65] += pt[:, qslice]^T @ V_aug, moving dim
    65 (vs 512 for O^T layout) -> half the AV PE time; the ones column
    of V_aug accumulates softmax sums per q on partitions. AV matmuls
    lag their exps by AV_LAG entries so the backlog fills the PE during
    the last chunk (no projection work left) and the final drain is
    head-ordered so heads 2/3 fill the pair-0 normalize window
  - normalize with per-partition reciprocal scalars (ACT/DVE split),
    PE-transpose X -> X^T via identity matmul, row-parallel Wo partial,
    one batched bf16 store per q-tile (f32 stores / per-512-col stores
    are DMA- resp. HWDGE-generation-bound in the tail)
Host: shards inputs (transposes q/kv once, bf16 cast), upcasts and sums
the 4 bf16 partials per batch in f32, +bo.

Self-contained: hardcodes all shapes; requires concourse + numpy + ml_dtypes.
"""

import os

import numpy as np
import ml_dtypes

import concourse.bass as bass  # noqa: F401  (bass types via bacc/tile)
import concourse.tile as tile
from concourse import bacc, mybir
from concourse import bass_utils

F32 = mybir.dt.float32
F32R = mybir.dt.float32r
BF16 = mybir.dt.bfloat16
EXP = mybir.ActivationFunctionType.Exp

B, NQ, NKV, D = 2, 512, 4096, 1024
HEADS, DH = 16, 64
SCALE = DH ** -0.5
N_CORES = 8
HPC = HEADS // (N_CORES // B)   # heads per core = 4
PAIRS = HPC // 2                # head pairs per core = 2
KO = D // 128                   # 8 contraction sub-chunks
# kv chunk sizes in 128-key groups: small first chunk so attention starts
# before the bulk of the prologue DMA lands; small last chunk to shorten
# the final ACT-bound exp wave
SIZES = [int(x) for x in os.environ.get("SIZES", "4,4,4,4,4,4,4,4").split(",")]
assert sum(SIZES) * 128 == NKV
NCHUNK = len(SIZES)
STARTS = [sum(SIZES[:i]) for i in range(NCHUNK)]
TOTAL_GROUPS = sum(SIZES)

KV_BUFS = int(os.environ.get("KV_BUFS", "3"))
AV_LAG = int(os.environ.get("AV_LAG", "88"))
AV_DRAIN = int(os.environ.get("AV_DRAIN", "20"))
PT_BUFS = int(os.environ.get("PT_BUFS", "118"))
V_BUFS = int(os.environ.get("V_BUFS", "7"))
PSS_BUFS = int(os.environ.get("PSS_BUFS", "2"))
_NC_CACHE = []


def _build_nc():
    nc = bacc.Bacc("TRN2", target_bir_lowering=False, debug=False,
                   num_devices=N_CORES)
    qT = nc.dram_tensor("qT", [D, NQ], BF16, kind="ExternalInput").ap()
    kvT = nc.dram_tensor("kvT", [D, NKV], BF16, kind="ExternalInput").ap()
    wq = nc.dram_tensor("wq", [D, HPC * DH], BF16, kind="ExternalInput").ap()
    wkv = nc.dram_tensor("wkv", [D, 2 * HPC * DH], BF16, kind="ExternalInput").ap()
    wo = nc.dram_tensor("wo", [HPC * DH, D], F32R, kind="ExternalInput").ap()
    bias = nc.dram_tensor("bias", [128, TOTAL_GROUPS], F32, kind="ExternalInput").ap()
    ident = nc.dram_tensor("ident", [128, 128], F32R, kind="ExternalInput").ap()
    # bf16 partial-output stores halve the tail DMA; host upcasts + sums the
    # 4 per-batch partials in f32, so the quantization hit is ~0.1% of out
    out = nc.dram_tensor("out", [NQ, D], BF16, kind="ExternalOutput").ap()
    dbg = None
    if os.environ.get("DEBUG_DUMP") == "1":
        dbg = nc.dram_tensor("dbg", [HPC, 128, 4 * (DH + 1)], F32,
                             kind="ExternalOutput").ap()

    qT_r = qT.rearrange("(ko p) n -> p ko n", p=128)
    kvT_r = kvT.rearrange("(ko p) n -> p ko n", p=128)
    wq_r = wq.rearrange("(ko p) m -> p ko m", p=128)
    wkv_r = wkv.rearrange("(ko p) (t m) -> p ko t m", p=128, t=2)
    wo_r = wo.rearrange("(ko p) n -> p ko n", p=128)

    with tile.TileContext(nc) as tc:
        with (
            tc.tile_pool(name="const", bufs=1) as cpool,
            tc.tile_pool(name="kv", bufs=KV_BUFS) as kv_pool,
            tc.tile_pool(name="kt", bufs=2) as kt_pool,
            tc.tile_pool(name="v", bufs=V_BUFS) as v_pool,
            tc.tile_pool(name="pt", bufs=PT_BUFS) as p_pool,
            tc.tile_pool(name="xp", bufs=4) as xp_pool,
            tc.tile_pool(name="ob", bufs=4) as o_pool,
            tc.tile_pool(name="psA", bufs=1, space="PSUM") as psA,
            tc.tile_pool(name="psV", bufs=1, space="PSUM") as psV,
            tc.tile_pool(name="psS", bufs=PSS_BUFS, space="PSUM") as psS,
            tc.tile_pool(name="psO", bufs=1, space="PSUM") as psO,
        ):
            wq_sb = cpool.tile([128, KO, HPC * DH], BF16, tag="wq")
            wkv_sb = cpool.tile([128, KO, 2, HPC * DH], BF16, tag="wkv")
            wo_sb = cpool.tile([128, PAIRS, D], F32R, tag="wo")
            qT_sb = cpool.tile([128, KO, NQ], BF16, tag="qT")
            bias_sb = cpool.tile([128, TOTAL_GROUPS], F32, tag="bias")
            ident_sb = cpool.tile([128, 128], F32R, tag="ident")
            qh_sb = cpool.tile([128, PAIRS, NQ], BF16, tag="qh")
            xT = [cpool.tile([128, NQ], F32R, tag=f"x{p}", name=f"xT{p}")
                  for p in range(PAIRS)]
            rsb = [cpool.tile([128, 4], F32, tag=f"r{h}", name=f"rsb{h}")
                   for h in range(HPC)]

            # warmup scratch: dummy matmuls placed into known prologue DMA
            # stalls keep the PE's p-state ramp alive (a gap drops the clock
            # to 1.2GHz for 3us of the following real matmuls)
            dsb = cpool.tile([128, 512], BF16, tag="dsb")
            nc.vector.memset(dsb[:], 0.0)
            warm_n = [0]

            def warm(n):
                for _ in range(n):
                    wps = psS.tile([128, 512], F32, tag="psS",
                                   name=f"warm{warm_n[0]}")
                    warm_n[0] += 1
                    nc.tensor.matmul(wps[:], dsb[0:128, 0:128], dsb[:, :],
                                     start=True, stop=True)

            # prologue DMAs, strip-granular and ordered exactly by first use:
            # Q projection streams first (per-ko, both pairs), then chunk-0
            # K projection ko-major, then V projection, then chunk 1
            kvc0 = kv_pool.tile([128, KO, SIZES[0] * 128], BF16, tag="kvc",
                                name="kvc0")
            kvc1 = kv_pool.tile([128, KO, SIZES[1] * 128], BF16, tag="kvc",
                                name="kvc1")
            nc.sync.dma_start(wq_sb[:, 0:4, :], wq_r[:, 0:4, :])
            nc.sync.dma_start(qT_sb[:, 0:2, :], qT_r[:, 0:2, :])
            nc.sync.dma_start(wq_sb[:, 4:KO, :], wq_r[:, 4:KO, :])
            nc.sync.dma_start(qT_sb[:, 2:4, :], qT_r[:, 2:4, :])
            nc.sync.dma_start(qT_sb[:, 4:KO, :], qT_r[:, 4:KO, :])
            nc.sync.dma_start(wkv_sb[:, 0:4, :, :], wkv_r[:, 0:4, :, :])
            nc.sync.dma_start(kvc0[:, 0:2, :], kvT_r[:, 0:2, 0:SIZES[0] * 128])
            nc.sync.dma_start(kvc0[:, 2:4, :], kvT_r[:, 2:4, 0:SIZES[0] * 128])
            nc.sync.dma_start(wkv_sb[:, 4:KO, :, :], wkv_r[:, 4:KO, :, :])
            nc.sync.dma_start(kvc0[:, 4:6, :], kvT_r[:, 4:6, 0:SIZES[0] * 128])
            nc.sync.dma_start(kvc0[:, 6:KO, :], kvT_r[:, 6:KO, 0:SIZES[0] * 128])
            nc.sync.dma_start(
                kvc1[:], kvT_r[:, :, STARTS[1] * 128:(STARTS[1] + SIZES[1]) * 128])
            nc.sync.dma_start(bias_sb[:], bias)
            nc.sync.dma_start(ident_sb[:], ident)

            def q_projection():
                # ko-major over both pairs so the PE consumes qT strips as
                # they stream in
                qp0 = psA.tile([128, NQ], F32, tag="psA", name="qp0")
                qp1 = psV.tile([128, NQ], F32, tag="psV", name="qp1")
                for ko in range(KO):
                    nc.tensor.matmul(qp0[:], wq_sb[:, ko, 0:128],
                                     qT_sb[:, ko, :], start=(ko == 0),
                                     stop=(ko == KO - 1))
                    nc.tensor.matmul(qp1[:], wq_sb[:, ko, 128:256],
                                     qT_sb[:, ko, :], start=(ko == 0),
                                     stop=(ko == KO - 1))
                nc.vector.tensor_copy(qh_sb[:, 0, :], qp0[:])
                nc.scalar.copy(qh_sb[:, 1, :], qp1[:])

            def proj_c0(kvc, ktc, vc):
                # chunk-0 projection is ko-major across 4 concurrent PSUM
                # accumulators (both K pairs + first two V subs) so the PE
                # consumes kv strips at streaming rate; V m2/m3 follow once
                # the whole chunk has landed
                kp0 = psA.tile([128, SIZES[0] * 128], F32, tag="psA", name="kp0_0")
                kp1 = psV.tile([128, SIZES[0] * 128], F32, tag="psV", name="kp0_1")
                n_ko_major = min(2, SIZES[0])
                vps = [psS.tile([128, HPC * DH], F32, tag="psS", name=f"vp0_{m}")
                       for m in range(n_ko_major)]
                for ko in range(KO):
                    first, last = ko == 0, ko == KO - 1
                    nc.tensor.matmul(kp0[:], wkv_sb[:, ko, 0, 0:128],
                                     kvc[:, ko, :], start=first, stop=last)
                    nc.tensor.matmul(kp1[:], wkv_sb[:, ko, 0, 128:256],
                                     kvc[:, ko, :], start=first, stop=last)
                    for m in range(n_ko_major):
                        nc.tensor.matmul(vps[m][:], kvc[:, ko, 128 * m:128 * (m + 1)],
                                         wkv_sb[:, ko, 1, :], start=first, stop=last)
                nc.vector.tensor_copy(ktc[:, 0, :], kp0[:])
                nc.scalar.copy(ktc[:, 1, :], kp1[:])
                for m in range(n_ko_major):
                    nc.vector.tensor_copy(
                        vc[:, m, :, 0:DH],
                        vps[m][:].rearrange("p (h d) -> p h d", h=HPC),
                    )
                for m in range(n_ko_major, SIZES[0]):
                    proj_v_sub(0, kvc, vc, m, pool=(psA, psV)[m % 2])

            # persistent O accumulators, one bank per head:
            # [128 q-partitions, 4 qtiles x (dh + ones-column sums)].
            # Zeroed explicitly: a start=True matmul resets the whole PSUM
            # bank, clobbering sibling qtile regions, so every AV matmul
            # accumulates (start=False) onto these zeros instead.
            opsum = [psO.tile([128, 4 * (DH + 1)], F32, tag=f"o{h}", name=f"opsum{h}")
                     for h in range(HPC)]
            for h in range(HPC):
                nc.vector.memset(opsum[h][:], 0.0)

            def load_chunk(c):
                if c == 0:
                    return kvc0
                if c == 1:
                    return kvc1
                kvc = kv_pool.tile([128, KO, SIZES[c] * 128], BF16, tag="kvc",
                                   name=f"kvc{c}")
                nc.sync.dma_start(
                    kvc[:],
                    kvT_r[:, :, STARTS[c] * 128:(STARTS[c] + SIZES[c]) * 128])
                return kvc

            def proj_k_pair(c, kvc, ktc, p):
                kp = psA.tile([128, SIZES[c] * 128], F32, tag="psA",
                              name=f"kp{c}_{p}")
                for ko in range(KO):
                    nc.tensor.matmul(
                        kp[:], wkv_sb[:, ko, 0, 128 * p:128 * (p + 1)],
                        kvc[:, ko, :], start=(ko == 0), stop=(ko == KO - 1),
                    )
                nc.vector.tensor_copy(ktc[:, p, :], kp[:])

            def proj_v_sub(c, kvc, vc, m, pool=None):
                pool = pool or psV
                vp = pool.tile([128, HPC * DH], F32, tag=pool.name, name=f"vp{c}_{m}")
                for ko in range(KO):
                    nc.tensor.matmul(
                        vp[:], kvc[:, ko, 128 * m:128 * (m + 1)],
                        wkv_sb[:, ko, 1, :], start=(ko == 0), stop=(ko == KO - 1),
                    )
                nc.vector.tensor_copy(
                    vc[:, m, :, 0:DH],
                    vp[:].rearrange("p (h d) -> p h d", h=HPC),
                )

            def alloc_proj_tiles(c):
                ktc = kt_pool.tile([128, PAIRS, SIZES[c] * 128], BF16, tag="ktc",
                                   name=f"ktc{c}")
                vc = v_pool.tile([128, SIZES[c], HPC, DH + 1], BF16, tag="vc",
                                 name=f"vc{c}")
                nc.vector.memset(vc[:, :, :, DH:DH + 1], 1.0)
                return ktc, vc

            def proj_pieces(c, kvc, ktc, vc):
                # closures, emitted spread across the previous chunk's slots
                ps = [lambda p=p: proj_k_pair(c, kvc, ktc, p) for p in range(PAIRS)]
                ps += [lambda m=m: proj_v_sub(c, kvc, vc, m)
                       for m in range(SIZES[c])]
                return ps

            av_pending = []

            def qk_exp_group(c, ktc, vc, s, mid_flush=None):
                g = STARTS[c] + s
                bias_ap = bias_sb[:, g:g + 1]
                for p in range(PAIRS):
                    sps = []
                    for half in range(2):  # row-tiled pair, K=64
                        lo, hi = 64 * half, 64 * (half + 1)
                        sp = psS.tile([128, NQ], F32, tag="psS", name=f"sp{c}_{s}_{p}_{half}")
                        nc.tensor.matmul(
                            sp[:], ktc[lo:hi, p, 128 * s:128 * (s + 1)],
                            qh_sb[lo:hi, p, :], start=True, stop=True,
                        )
                        sps.append(sp)
                    for half, sp in enumerate(sps):
                        h = 2 * p + half
                        pt = p_pool.tile([128, NQ], BF16, tag="pt", name=f"pt{c}_{s}_{p}_{half}")
                        nc.scalar.activation(
                            pt[:], sp[:], EXP, bias=bias_ap, scale=SCALE,
                        )
                        av_pending.append((c, s, h, vc, pt))
                    if p == 0 and mid_flush is not None:
                        # keep the PE fed while the second pair's QK matmuls
                        # wait on PSUM banks held by in-flight exps
                        flush_av(mid_flush)

            def emit_av(entry):
                c, s, h, vc, pt = entry
                for m in range(4):
                    nc.tensor.matmul(
                        opsum[h][:, 65 * m:65 * (m + 1)],
                        pt[:, 128 * m:128 * (m + 1)],
                        vc[:, s, h, :],
                        start=False,
                        stop=(STARTS[c] + s == TOTAL_GROUPS - 1),
                        skip_group_check=True,
                    )

            def flush_av(upto):
                # emit AV matmuls (O-layout, moving dim 65) for groups that
                # have lagged enough behind their exp
                while av_pending and len(av_pending) > upto:
                    emit_av(av_pending.pop(0))

            # prologue: Q projection (streams qT), then chunk-0 projection,
            # with warmup dummies covering the DMA-wait gaps
            kvc_cur = load_chunk(0)
            ktc_cur, vc_cur = alloc_proj_tiles(0)
            warm(int(os.environ.get("WARM0", "8")))
            q_projection()
            warm(int(os.environ.get("WARM1", "0")))
            proj_c0(kvc_cur, ktc_cur, vc_cur)
            warm(int(os.environ.get("WARM2", "0")))

            # steady state: attention(c) interleaved with projections(c+1)
            # spread across c's attention slots; AV lags behind its exp to
            # hide ACT->PE sem latency. kv chunk DMAs are issued two
            # iterations ahead of their projection. The second-to-last chunk
            # has spare slot time (small next chunk) and the last chunk has
            # none at all, so the AV backlog drains through their ACT-bound
            # windows on a stepped schedule.
            kvc_next = {0: kvc0, 1: kvc1}
            drain7 = int(os.environ.get("AV_DRAIN7", "0"))
            for c in range(NCHUNK):
                if c + 2 < NCHUNK:
                    kvc_next[c + 2] = load_chunk(c + 2)
                if c == 0:
                    nc.sync.dma_start(wo_sb[:], wo_r)
                pieces = []
                if c + 1 < NCHUNK:
                    kvc_nxt = kvc_next[c + 1]
                    ktc_nxt, vc_nxt = alloc_proj_tiles(c + 1)
                    pieces = proj_pieces(c + 1, kvc_nxt, ktc_nxt, vc_nxt)
                slots = SIZES[c]
                npieces = len(pieces)
                if slots == 4 and npieces == 6:
                    bounds = [0, 1, 2, 4, 6]  # K0 | K1 | V0 V1 | V2 V3
                else:
                    bounds = [npieces * s // slots for s in range(slots + 1)]
                for s in range(slots):
                    if c == NCHUNK - 1:
                        # final chunk: steep drain, mid-group flush keeps the
                        # PE fed while the last exps hold the score banks
                        target = max(0, AV_LAG - drain7 * 4
                                     - AV_DRAIN * (s + 1))
                        mid = AV_DRAIN // 2 if s < slots - 1 else 2
                        qk_exp_group(c, ktc_cur, vc_cur, s,
                                     mid_flush=target + mid)
                        flush_av(target)
                        continue
                    qk_exp_group(c, ktc_cur, vc_cur, s)
                    if c == NCHUNK - 2:
                        flush_av(max(0, AV_LAG - drain7 * (s + 1)))
                    else:
                        flush_av(AV_LAG)
                    for piece in pieces[bounds[s]:bounds[s + 1]]:
                        piece()
                if c + 1 < NCHUNK:
                    kvc_cur, ktc_cur, vc_cur = kvc_nxt, ktc_nxt, vc_nxt

            # head-ordered final drain: finish heads 0/1 first so their
            # normalize chain can start while heads 2/3's AV matmuls fill
            # the PE gaps between pair-0 transposes
            rest = sorted(av_pending, key=lambda e: e[2])
            av_pending.clear()
            lo_avs = [e for e in rest if e[2] < 2]
            hi_avs = [e for e in rest if e[2] >= 2]
            for e in lo_avs:
                emit_av(e)
            hi_batches = [hi_avs[i::4] for i in range(4)]

            # normalize: reciprocal of the ones-column sums (per-partition),
            # X = O * (1/sum) via per-partition scalar on ACT/DVE, then
            # PE-transpose X -> X^T via identity matmul
            if dbg is not None:
                for batch in hi_batches:
                    for e in batch:
                        emit_av(e)
                hi_batches = [[], [], [], []]
                for h in range(HPC):
                    dsb = o_pool.tile([128, 4 * (DH + 1)], F32, tag="dsb",
                                      name=f"dsb{h}")
                    nc.vector.tensor_copy(dsb[:], opsum[h][:])
                    nc.sync.dma_start(dbg[h], dsb[:])

            wo_pools = [psA, psV]

            def emit_wo(m):
                # one batched [128, 1024] bf16 store per q-tile: small stores
                # are HWDGE-generation-bound (632ns each), batching amortizes.
                # The final q-tile stores per-half so the end-of-kernel drain
                # only waits on a 512-column transfer.
                osb = o_pool.tile([128, D], BF16, tag="osb", name=f"osb{m}")
                for n in range(D // 512):
                    pool_w = wo_pools[n]
                    wp = pool_w.tile([128, 512], F32, tag=pool_w.name, name=f"wp{m}_{n}")
                    for p in range(PAIRS):
                        nc.tensor.matmul(
                            wp[:], xT[p][:, 128 * m:128 * (m + 1)],
                            wo_sb[:, p, 512 * n:512 * (n + 1)],
                            start=(p == 0), stop=(p == PAIRS - 1),
                        )
                    if n % 2 == 0:
                        nc.vector.tensor_copy(osb[:, 0:512], wp[:])
                    else:
                        nc.scalar.copy(osb[:, 512:D], wp[:])
                    if m == 3:
                        nc.sync.dma_start(
                            out[128 * m:128 * (m + 1), 512 * n:512 * (n + 1)],
                            osb[:, 512 * n:512 * (n + 1)])
                if m < 3:
                    nc.sync.dma_start(out[128 * m:128 * (m + 1), :], osb[:])

            # pair-major tail: pair 0's heads finish their exps/AV ~1.2us
            # before pair 1's, so normalize+transpose pair 0 while pair 1's
            # exps drain; Wo for qtile m follows pair 1's transpose of m.
            # TAIL_MODE=m emits qtile-major instead (AV drain upfront).
            def norm_unit(p, m):
                hA, hB = 2 * p, 2 * p + 1
                xp_sb = xp_pool.tile([128, 128], F32R, tag="xp", name=f"xp{m}_{p}")
                nc.scalar.mul(
                    xp_sb[:, 0:DH], opsum[hA][:, 65 * m:65 * m + DH],
                    rsb[hA][:, m:m + 1],
                )
                nc.vector.tensor_scalar_mul(
                    xp_sb[:, DH:128], opsum[hB][:, 65 * m:65 * m + DH],
                    rsb[hB][:, m:m + 1],
                )
                xtp = psS.tile([128, 128], F32R, tag="psS", name=f"xtp{m}_{p}")
                nc.tensor.transpose(xtp[:], xp_sb[:], ident_sb[:])
                if m % 2 == 0:
                    nc.vector.tensor_copy(xT[p][:, 128 * m:128 * (m + 1)], xtp[:])
                else:
                    nc.scalar.copy(xT[p][:, 128 * m:128 * (m + 1)], xtp[:])

            def recips(p):
                for hh in (2 * p, 2 * p + 1):
                    sums = opsum[hh].rearrange("p (m e) -> p m e", e=DH + 1)[:, :, DH]
                    nc.vector.reciprocal(rsb[hh][:], sums)

            if os.environ.get("TAIL_MODE", "m") == "m":
                for batch in hi_batches:
                    for e in batch:
                        emit_av(e)
                recips(0)
                recips(1)
                for m in range(4):
                    norm_unit(0, m)
                    norm_unit(1, m)
                    emit_wo(m)
            else:
                for p in range(PAIRS):
                    recips(p)
                    for m in range(4):
                        norm_unit(p, m)
                        if p == 0:
                            for e in hi_batches[m]:
                                emit_av(e)
                        else:
                            emit_wo(m)

    nc.compile()
    return nc


def _get_nc():
    if not _NC_CACHE:
        _NC_CACHE.append(_build_nc())
    return _NC_CACHE[0]


LAST_RESULTS = None


def kernel(q, kv, mask, Wq, Wkv, Wo, bo):
    global LAST_RESULTS
    q = np.asarray(q, dtype=np.float32)
    kv = np.asarray(kv, dtype=np.float32)
    mask = np.asarray(mask)
    Wq = np.asarray(Wq, dtype=np.float32)
    Wkv = np.asarray(Wkv, dtype=np.float32)
    Wo = np.asarray(Wo, dtype=np.float32)
    bo = np.asarray(bo, dtype=np.float32)

    bf16 = ml_dtypes.bfloat16
    inner = HEADS * DH
    qT = [np.ascontiguousarray(q[b].T.astype(bf16)) for b in range(B)]
    kvT = [np.ascontiguousarray(kv[b].T.astype(bf16)) for b in range(B)]
    ident = np.eye(128, dtype=np.float32)
    bias = []
    for b in range(B):
        bb = np.where(mask[b], 0.0, -30000.0).astype(np.float32)
        bias.append(np.ascontiguousarray(bb.reshape(TOTAL_GROUPS, 128).T))

    in_maps = []
    for i in range(N_CORES):
        b, g = divmod(i, N_CORES // B)
        cs = slice(HPC * DH * g, HPC * DH * (g + 1))
        in_maps.append({
            "qT": qT[b],
            "kvT": kvT[b],
            "wq": np.ascontiguousarray(Wq[:, cs].astype(bf16)),
            "wkv": np.ascontiguousarray(np.concatenate(
                [Wkv[:, cs], Wkv[:, inner:][:, cs]], axis=1).astype(bf16)),
            "wo": np.ascontiguousarray(Wo[cs, :]),
            "bias": bias[b],
            "ident": ident,
        })

    nc = _get_nc()
    res = bass_utils.run_bass_kernel_spmd(
        nc, in_maps, core_ids=list(range(N_CORES)))
    LAST_RESULTS = res

    gpb = N_CORES // B
    out = np.zeros((B, NQ, D), np.float32)
    for b in range(B):
        acc = res.results[b * gpb]["out"].astype(np.float32).copy()
        for g in range(1, gpb):
            acc += res.results[b * gpb + g]["out"].astype(np.float32)
        out[b] = acc + bo[None, :]
    return out
